# revision 8
# baseline (speedup 1.0000x reference)
"""v9: circular-padded input + dynamic-offset full-partition DMAs.

Problem: x [64, 3, 512, 512] f32, shifts [64, 2] int32 in [-16, 16].
out[b, c, h, w] = x[b, c, (h - shifts[b,0]) % 512, (w - shifts[b,1]) % 512]

Pure data parallel over batch (8 per core). Host-side, each channel is
circular-padded by MAX_SHIFT=16 on every border (544x544) and stored as
one flat row of a [24, 296000] tensor (64 tail-pad elems so the widest
dynamic window stays in bounds). BOTH rolls then become a plain window
read at element offset (16-sh)*544 + (16-sw) -- no wraparound pieces,
no If chains. Each (batch, channel) is ONE fixed-shape [128, 4, 512]
load DMA whose DRAM offset is a single register value; the store is a
fully static contiguous DMA.

Why [128, ...] everywhere: the HWDGE fans one DMA's descriptors across
n SDMA engines where n = largest divisor <= 16 of the partition count
(measured: 128 parts -> 16 engines at 341 GB/s; 42 -> 14; 41 (prime) ->
~1 engine at 24 GB/s). The previous design's H-roll pieces had 36-43
partition bodies -- a gcd lottery that serialized ~70% of load bytes on
one engine (761 us total vs ~141 us HBM roofline).

Engine assignment: each dynamic-offset DMA permanently consumes a few
sequencer registers at trace time (offset + bounds-check lowering), and
an engine has only 49 -- 24 dynamic loads on one engine exhausts the
file. So the dynamic loads are split across the two HWDGE sequencers
(sync: batches 0-3, scalar: batches 4-7, own register files) and the
static stores go through gpsimd (SWDGE, measured 377 GB/s on
[128, x, 512] shapes). Offsets are computed with in-place reg ALU ops
on two reused registers (reg_alu with an int immediate also leaks a
const register per call, so 16 and 544 are hoisted into registers).

SBUF layout per slot: [128, 3, 4, 512]: tile[p, c*2048 + j*512 + w]
holds out[b, c, p*4 + j, w].
"""

from contextlib import ExitStack

import numpy as np

import concourse.bass as bass
import concourse.mybir as mybir
from bass_rust import RegisterHandles, make_scalar_value
from concourse.bass_utils import run_bass_kernel_spmd

B_TOTAL, C, H, W = 64, 3, 512, 512
N_CORES = 8
B = B_TOTAL // N_CORES
MAX_SHIFT = 16
PAD = 2 * MAX_SHIFT  # 32
HP, WP = H + PAD, W + PAD  # 544, 544
CH_ELEMS = HP * WP  # 295936
CH_STRIDE = CH_ELEMS + 64  # 296000, tail pad keeps max window in bounds
P = 128
JH = H // P  # 4 rows of a channel per partition
WIN = P * JH * WP  # 278528: window covering 512 padded rows
MAX_OFF = PAD * WP + PAD  # 17440
NBUF = 6

LOADS_PER_BATCH = C
STORES_PER_BATCH = C


def build_kernel():
    nc = bass.Bass()
    x = nc.dram_tensor("x", [B * C, CH_STRIDE], mybir.dt.float32, kind="ExternalInput")
    shifts = nc.dram_tensor("shifts", [B, 2], mybir.dt.int32, kind="ExternalInput")
    out = nc.dram_tensor("out", [B, C, H, W], mybir.dt.float32, kind="ExternalOutput")

    CW = JH * W  # 2048 elems per channel per partition

    with (
        nc.sbuf_tensor([P, NBUF, C * CW], mybir.dt.float32) as tiles,
        nc.sbuf_tensor([1, B * 2], mybir.dt.int32) as sb_shifts,
        nc.semaphore("pre_sem") as pre_sem,
        ExitStack() as stack,
    ):
        # per (slot, channel) semaphores: stores start as soon as their own
        # 1 MB channel lands (not the whole 3 MB batch), and channel loads
        # only wait for the matching channel's old store -- trims the
        # pipeline ramp and tail by ~2 channels' worth of transfer time.
        load_sems = [
            [
                stack.enter_context(nc.semaphore(f"load_sem{s}_{c}"))
                for c in range(C)
            ]
            for s in range(NBUF)
        ]
        store_sems = [
            [
                stack.enter_context(nc.semaphore(f"store_sem{s}_{c}"))
                for c in range(C)
            ]
            for s in range(NBUF)
        ]
        block = stack.enter_context(nc.Block())

        def emit_load(eng, b, rb):
            s = b % NBUF
            tile_s = tiles[:, s]
            for c in range(C):
                if b >= NBUF:
                    eng.wait_ge(store_sems[s][c], 16 * (b // NBUF))
                win = x[b * C + c, bass.ds(rb, WIN)]
                src = win.rearrange("(p j w) -> p j w", j=JH, w=WP)[:, :, 0:W]
                eng.dma_start(tile_s[:, c * CW : (c + 1) * CW], src).then_inc(
                    load_sems[s][c], 16
                )

        def emit_store(eng, b):
            s = b % NBUF
            tile_s = tiles[:, s]
            for c in range(C):
                eng.wait_ge(load_sems[s][c], 16 * (b // NBUF + 1))
                eng.dma_start(
                    out[b, c].rearrange("(p j) w -> p (j w)", j=JH),
                    tile_s[:, c * CW : (c + 1) * CW],
                ).then_inc(store_sems[s][c], 16)

        def emit_half(eng, my_batches):
            # Interleave loads and stores on ONE engine/ring: SDMA engines
            # round-robin between queues at packet granularity, so the
            # load:store bandwidth split matches the queue mix. Two load
            # queues + one store queue (v10) gave loads 2/3 of bandwidth:
            # all loads finished at t=100us, stores drained alone for 45us.
            # Each engine storing its own batches (lag 1 batch) keeps every
            # ring 50/50 load/store.
            with (
                eng.register("r_off") as r_off,
                eng.register("r_sw") as r_sw,
                eng.register("r_c16") as r_c16,
                eng.register("r_cWP") as r_cWP,
            ):
                eng.reg_mov(r_c16, MAX_SHIFT)
                eng.reg_mov(r_cWP, WP)
                for i, b in enumerate(my_batches):
                    eng.reg_load(r_off, sb_shifts[0:1, 2 * b : 2 * b + 1])
                    eng.reg_load(r_sw, sb_shifts[0:1, 2 * b + 1 : 2 * b + 2])
                    # r_off = (16 - sh) * 544 + (16 - sw)
                    eng.reg_sub(r_off, r_c16, r_off)
                    eng.reg_sub(r_sw, r_c16, r_sw)
                    eng.reg_mul(r_off, r_off, r_cWP)
                    eng.reg_add(r_off, r_off, r_sw)
                    rb = make_scalar_value(
                        RegisterHandles([r_off]), min_val=0, max_val=MAX_OFF
                    )
                    emit_load(eng, b, rb)
                    if i >= 1:
                        emit_store(eng, my_batches[i - 1])
                emit_store(eng, my_batches[-1])
            for b in my_batches:
                s = b % NBUF
                for c in range(C):
                    eng.wait_ge(store_sems[s][c], 16 * (b // NBUF + 1))

        @block.sync
        def _(sync):
            sync.dma_start(
                sb_shifts[0:1, :], shifts.rearrange("b s -> (b s)")[None, :]
            ).then_inc(pre_sem, 16)
            sync.wait_ge(pre_sem, 16)
            emit_half(sync, list(range(0, B // 2)))

        @block.scalar
        def _(scalar):
            scalar.wait_ge(pre_sem, 16)
            emit_half(scalar, list(range(B // 2, B)))

    return nc


_NC_CACHE = None


def _get_nc():
    global _NC_CACHE
    if _NC_CACHE is None:
        _NC_CACHE = build_kernel()
    return _NC_CACHE


def _pad_input(x: np.ndarray) -> np.ndarray:
    """[64, 3, 512, 512] -> [64*3, 296000]: per-channel circular 16-px
    border (544x544) flattened, with 64 tail-pad elems per channel."""
    xp = np.pad(
        x,
        ((0, 0), (0, 0), (MAX_SHIFT, MAX_SHIFT), (MAX_SHIFT, MAX_SHIFT)),
        mode="wrap",
    ).reshape(B_TOTAL * C, CH_ELEMS)
    outp = np.zeros((B_TOTAL * C, CH_STRIDE), dtype=np.float32)
    outp[:, :CH_ELEMS] = xp
    return outp


def kernel(x: np.ndarray, shifts: np.ndarray) -> np.ndarray:
    assert x.shape == (B_TOTAL, C, H, W), x.shape
    assert shifts.shape == (B_TOTAL, 2), shifts.shape
    x = np.ascontiguousarray(x, dtype=np.float32)
    shifts = np.ascontiguousarray(shifts, dtype=np.int32)
    x_pad = _pad_input(x)

    in_maps = [
        {
            "x": x_pad[i * B * C : (i + 1) * B * C],
            "shifts": shifts[i * B : (i + 1) * B],
        }
        for i in range(N_CORES)
    ]
    res = run_bass_kernel_spmd(_get_nc(), in_maps, list(range(N_CORES)))
    return np.concatenate(
        [res.results[i]["out"] for i in range(N_CORES)], axis=0
    ).astype(np.float32)


# revision 9
# speedup vs baseline: 1.0029x; 1.0029x over previous
"""v9: circular-padded input + dynamic-offset full-partition DMAs.

Problem: x [64, 3, 512, 512] f32, shifts [64, 2] int32 in [-16, 16].
out[b, c, h, w] = x[b, c, (h - shifts[b,0]) % 512, (w - shifts[b,1]) % 512]

Pure data parallel over batch (8 per core). Host-side, each channel is
circular-padded by MAX_SHIFT=16 on every border (544x544) and stored as
one flat row of a [24, 296000] tensor (64 tail-pad elems so the widest
dynamic window stays in bounds). BOTH rolls then become a plain window
read at element offset (16-sh)*544 + (16-sw) -- no wraparound pieces,
no If chains. Each (batch, channel) is ONE fixed-shape [128, 4, 512]
load DMA whose DRAM offset is a single register value; the store is a
fully static contiguous DMA.

Why [128, ...] everywhere: the HWDGE fans one DMA's descriptors across
n SDMA engines where n = largest divisor <= 16 of the partition count
(measured: 128 parts -> 16 engines at 341 GB/s; 42 -> 14; 41 (prime) ->
~1 engine at 24 GB/s). The previous design's H-roll pieces had 36-43
partition bodies -- a gcd lottery that serialized ~70% of load bytes on
one engine (761 us total vs ~141 us HBM roofline).

Engine assignment: each dynamic-offset DMA permanently consumes a few
sequencer registers at trace time (offset + bounds-check lowering), and
an engine has only 49 -- 24 dynamic loads on one engine exhausts the
file. So the dynamic loads are split across the two HWDGE sequencers
(sync: batches 0-3, scalar: batches 4-7, own register files) and the
static stores go through gpsimd (SWDGE, measured 377 GB/s on
[128, x, 512] shapes). Offsets are computed with in-place reg ALU ops
on two reused registers (reg_alu with an int immediate also leaks a
const register per call, so 16 and 544 are hoisted into registers).

SBUF layout per slot: [128, 3, 4, 512]: tile[p, c*2048 + j*512 + w]
holds out[b, c, p*4 + j, w].
"""

from contextlib import ExitStack

import numpy as np

import concourse.bass as bass
import concourse.mybir as mybir
from bass_rust import RegisterHandles, make_scalar_value
from concourse.bass_utils import run_bass_kernel_spmd

B_TOTAL, C, H, W = 64, 3, 512, 512
N_CORES = 8
B = B_TOTAL // N_CORES
MAX_SHIFT = 16
PAD = 2 * MAX_SHIFT  # 32
HP, WP = H + PAD, W + PAD  # 544, 544
CH_ELEMS = HP * WP  # 295936
CH_STRIDE = CH_ELEMS + 64  # 296000, tail pad keeps max window in bounds
P = 128
JH = H // P  # 4 rows of a channel per partition
WIN = P * JH * WP  # 278528: window covering 512 padded rows
MAX_OFF = PAD * WP + PAD  # 17440
NBUF = 8  # one slot per batch: no slot reuse, no cross-engine waits

LOADS_PER_BATCH = C
STORES_PER_BATCH = C


def build_kernel():
    nc = bass.Bass()
    x = nc.dram_tensor("x", [B * C, CH_STRIDE], mybir.dt.float32, kind="ExternalInput")
    shifts = nc.dram_tensor("shifts", [B, 2], mybir.dt.int32, kind="ExternalInput")
    out = nc.dram_tensor("out", [B, C, H, W], mybir.dt.float32, kind="ExternalOutput")

    CW = JH * W  # 2048 elems per channel per partition

    with (
        nc.sbuf_tensor([P, NBUF, C * CW], mybir.dt.float32) as tiles,
        nc.sbuf_tensor([1, B * 2], mybir.dt.int32) as sb_shifts,
        nc.semaphore("pre_sem") as pre_sem,
        ExitStack() as stack,
    ):
        # per (slot, channel) semaphores: stores start as soon as their own
        # 1 MB channel lands (not the whole 3 MB batch), and channel loads
        # only wait for the matching channel's old store -- trims the
        # pipeline ramp and tail by ~2 channels' worth of transfer time.
        load_sems = [
            [
                stack.enter_context(nc.semaphore(f"load_sem{s}_{c}"))
                for c in range(C)
            ]
            for s in range(NBUF)
        ]
        store_sems = [
            [
                stack.enter_context(nc.semaphore(f"store_sem{s}_{c}"))
                for c in range(C)
            ]
            for s in range(NBUF)
        ]
        block = stack.enter_context(nc.Block())

        def emit_load(eng, b, rb):
            s = b % NBUF
            tile_s = tiles[:, s]
            for c in range(C):
                if b >= NBUF:
                    eng.wait_ge(store_sems[s][c], 16 * (b // NBUF))
                win = x[b * C + c, bass.ds(rb, WIN)]
                src = win.rearrange("(p j w) -> p j w", j=JH, w=WP)[:, :, 0:W]
                eng.dma_start(tile_s[:, c * CW : (c + 1) * CW], src).then_inc(
                    load_sems[s][c], 16
                )

        def emit_store(eng, b):
            s = b % NBUF
            tile_s = tiles[:, s]
            for c in range(C):
                eng.wait_ge(load_sems[s][c], 16 * (b // NBUF + 1))
                eng.dma_start(
                    out[b, c].rearrange("(p j) w -> p (j w)", j=JH),
                    tile_s[:, c * CW : (c + 1) * CW],
                ).then_inc(store_sems[s][c], 16)

        def emit_half(eng, my_batches):
            # Interleave loads and stores on ONE engine/ring: SDMA engines
            # round-robin between queues at packet granularity, so the
            # load:store bandwidth split matches the queue mix. Two load
            # queues + one store queue (v10) gave loads 2/3 of bandwidth:
            # all loads finished at t=100us, stores drained alone for 45us.
            # Each engine storing its own batches (lag 1 batch) keeps every
            # ring 50/50 load/store.
            with (
                eng.register("r_off") as r_off,
                eng.register("r_sw") as r_sw,
                eng.register("r_c16") as r_c16,
                eng.register("r_cWP") as r_cWP,
            ):
                eng.reg_mov(r_c16, MAX_SHIFT)
                eng.reg_mov(r_cWP, WP)
                for i, b in enumerate(my_batches):
                    eng.reg_load(r_off, sb_shifts[0:1, 2 * b : 2 * b + 1])
                    eng.reg_load(r_sw, sb_shifts[0:1, 2 * b + 1 : 2 * b + 2])
                    # r_off = (16 - sh) * 544 + (16 - sw)
                    eng.reg_sub(r_off, r_c16, r_off)
                    eng.reg_sub(r_sw, r_c16, r_sw)
                    eng.reg_mul(r_off, r_off, r_cWP)
                    eng.reg_add(r_off, r_off, r_sw)
                    rb = make_scalar_value(
                        RegisterHandles([r_off]), min_val=0, max_val=MAX_OFF
                    )
                    emit_load(eng, b, rb)
                    if i >= 1:
                        emit_store(eng, my_batches[i - 1])
                emit_store(eng, my_batches[-1])
            for b in my_batches:
                s = b % NBUF
                for c in range(C):
                    eng.wait_ge(store_sems[s][c], 16 * (b // NBUF + 1))

        @block.sync
        def _(sync):
            sync.dma_start(
                sb_shifts[0:1, :], shifts.rearrange("b s -> (b s)")[None, :]
            ).then_inc(pre_sem, 16)
            sync.wait_ge(pre_sem, 16)
            emit_half(sync, list(range(0, B // 2)))

        @block.scalar
        def _(scalar):
            scalar.wait_ge(pre_sem, 16)
            emit_half(scalar, list(range(B // 2, B)))

    return nc


_NC_CACHE = None


def _get_nc():
    global _NC_CACHE
    if _NC_CACHE is None:
        _NC_CACHE = build_kernel()
    return _NC_CACHE


def _pad_input(x: np.ndarray) -> np.ndarray:
    """[64, 3, 512, 512] -> [64*3, 296000]: per-channel circular 16-px
    border (544x544) flattened, with 64 tail-pad elems per channel."""
    xp = np.pad(
        x,
        ((0, 0), (0, 0), (MAX_SHIFT, MAX_SHIFT), (MAX_SHIFT, MAX_SHIFT)),
        mode="wrap",
    ).reshape(B_TOTAL * C, CH_ELEMS)
    outp = np.zeros((B_TOTAL * C, CH_STRIDE), dtype=np.float32)
    outp[:, :CH_ELEMS] = xp
    return outp


def kernel(x: np.ndarray, shifts: np.ndarray) -> np.ndarray:
    assert x.shape == (B_TOTAL, C, H, W), x.shape
    assert shifts.shape == (B_TOTAL, 2), shifts.shape
    x = np.ascontiguousarray(x, dtype=np.float32)
    shifts = np.ascontiguousarray(shifts, dtype=np.int32)
    x_pad = _pad_input(x)

    in_maps = [
        {
            "x": x_pad[i * B * C : (i + 1) * B * C],
            "shifts": shifts[i * B : (i + 1) * B],
        }
        for i in range(N_CORES)
    ]
    res = run_bass_kernel_spmd(_get_nc(), in_maps, list(range(N_CORES)))
    return np.concatenate(
        [res.results[i]["out"] for i in range(N_CORES)], axis=0
    ).astype(np.float32)


# revision 15
# speedup vs baseline: 1.0097x; 1.0068x over previous
"""v9: circular-padded input + dynamic-offset full-partition DMAs.

Problem: x [64, 3, 512, 512] f32, shifts [64, 2] int32 in [-16, 16].
out[b, c, h, w] = x[b, c, (h - shifts[b,0]) % 512, (w - shifts[b,1]) % 512]

Pure data parallel over batch (8 per core). Host-side, each channel is
circular-padded by MAX_SHIFT=16 on every border (544x544) and stored as
one flat row of a [24, 296000] tensor (64 tail-pad elems so the widest
dynamic window stays in bounds). BOTH rolls then become a plain window
read at element offset (16-sh)*544 + (16-sw) -- no wraparound pieces,
no If chains. Each (batch, channel) is ONE fixed-shape [128, 4, 512]
load DMA whose DRAM offset is a single register value; the store is a
fully static contiguous DMA.

Why [128, ...] everywhere: the HWDGE fans one DMA's descriptors across
n SDMA engines where n = largest divisor <= 16 of the partition count
(measured: 128 parts -> 16 engines at 341 GB/s; 42 -> 14; 41 (prime) ->
~1 engine at 24 GB/s). The previous design's H-roll pieces had 36-43
partition bodies -- a gcd lottery that serialized ~70% of load bytes on
one engine (761 us total vs ~141 us HBM roofline).

Engine assignment: each dynamic-offset DMA permanently consumes a few
sequencer registers at trace time (offset + bounds-check lowering), and
an engine has only 49 -- 24 dynamic loads on one engine exhausts the
file. So the dynamic loads are split across the two HWDGE sequencers
(sync: batches 0-3, scalar: batches 4-7, own register files) and the
static stores go through gpsimd (SWDGE, measured 377 GB/s on
[128, x, 512] shapes). Offsets are computed with in-place reg ALU ops
on two reused registers (reg_alu with an int immediate also leaks a
const register per call, so 16 and 544 are hoisted into registers).

SBUF layout per slot: [128, 3, 4, 512]: tile[p, c*2048 + j*512 + w]
holds out[b, c, p*4 + j, w].
"""

from contextlib import ExitStack

import numpy as np

import concourse.bass as bass
import concourse.mybir as mybir
from bass_rust import RegisterHandles, make_scalar_value
from concourse.bass_utils import run_bass_kernel_spmd

B_TOTAL, C, H, W = 64, 3, 512, 512
N_CORES = 8
B = B_TOTAL // N_CORES
MAX_SHIFT = 16
PAD = 2 * MAX_SHIFT  # 32
HP, WP = H + PAD, W + PAD  # 544, 544
CH_ELEMS = HP * WP  # 295936
CH_STRIDE = CH_ELEMS + 64  # 296000, tail pad keeps max window in bounds
P = 128
JH = H // P  # 4 rows of a channel per partition
WIN = P * JH * WP  # 278528: window covering 512 padded rows
MAX_OFF = PAD * WP + PAD  # 17440
NBUF = 8  # one slot per batch: no slot reuse, no cross-engine waits

LOADS_PER_BATCH = C
STORES_PER_BATCH = C


def build_kernel():
    nc = bass.Bass()
    x = nc.dram_tensor("x", [B * C, CH_STRIDE], mybir.dt.float32, kind="ExternalInput")
    shifts = nc.dram_tensor("shifts", [B, 2], mybir.dt.int32, kind="ExternalInput")
    out = nc.dram_tensor("out", [B, C, H, W], mybir.dt.float32, kind="ExternalOutput")

    CW = JH * W  # 2048 elems per channel per partition

    with (
        nc.sbuf_tensor([P, NBUF, C * CW], mybir.dt.float32) as tiles,
        nc.sbuf_tensor([1, B * 2], mybir.dt.int32) as sb_shifts,
        nc.semaphore("pre_sem") as pre_sem,
        ExitStack() as stack,
    ):
        # per (slot, channel) semaphores: stores start as soon as their own
        # 1 MB channel lands (not the whole 3 MB batch), and channel loads
        # only wait for the matching channel's old store -- trims the
        # pipeline ramp and tail by ~2 channels' worth of transfer time.
        load_sems = [
            [
                stack.enter_context(nc.semaphore(f"load_sem{s}_{c}"))
                for c in range(C)
            ]
            for s in range(NBUF)
        ]
        store_sems = [
            [
                stack.enter_context(nc.semaphore(f"store_sem{s}_{c}"))
                for c in range(C)
            ]
            for s in range(NBUF)
        ]
        block = stack.enter_context(nc.Block())

        def emit_load(eng, b, rb):
            s = b % NBUF
            tile_s = tiles[:, s]
            for c in range(C):
                if b >= NBUF:
                    eng.wait_ge(store_sems[s][c], 16 * (b // NBUF))
                win = x[b * C + c, bass.ds(rb, WIN)]
                src = win.rearrange("(p j w) -> p j w", j=JH, w=WP)[:, :, 0:W]
                eng.dma_start(tile_s[:, c * CW : (c + 1) * CW], src).then_inc(
                    load_sems[s][c], 16
                )

        def emit_store(eng, b):
            s = b % NBUF
            tile_s = tiles[:, s]
            for c in range(C):
                eng.wait_ge(load_sems[s][c], 16 * (b // NBUF + 1))
                eng.dma_start(
                    out[b, c].rearrange("(p j) w -> p (j w)", j=JH),
                    tile_s[:, c * CW : (c + 1) * CW],
                ).then_inc(store_sems[s][c], 16)

        def emit_half(eng, my_batches):
            # Interleave loads and stores on ONE engine/ring: SDMA engines
            # round-robin between queues at packet granularity, so the
            # load:store bandwidth split matches the queue mix. Two load
            # queues + one store queue (v10) gave loads 2/3 of bandwidth:
            # all loads finished at t=100us, stores drained alone for 45us.
            # Each engine storing its own batches (lag 1 batch) keeps every
            # ring 50/50 load/store.
            with (
                eng.register("r_off") as r_off,
                eng.register("r_sw") as r_sw,
                eng.register("r_c16") as r_c16,
                eng.register("r_cWP") as r_cWP,
            ):
                eng.reg_mov(r_c16, MAX_SHIFT)
                eng.reg_mov(r_cWP, WP)
                for i, b in enumerate(my_batches):
                    eng.reg_load(r_off, sb_shifts[0:1, 2 * b : 2 * b + 1])
                    eng.reg_load(r_sw, sb_shifts[0:1, 2 * b + 1 : 2 * b + 2])
                    # r_off = (16 - sh) * 544 + (16 - sw)
                    eng.reg_sub(r_off, r_c16, r_off)
                    eng.reg_sub(r_sw, r_c16, r_sw)
                    eng.reg_mul(r_off, r_off, r_cWP)
                    eng.reg_add(r_off, r_off, r_sw)
                    rb = make_scalar_value(
                        RegisterHandles([r_off]), min_val=0, max_val=MAX_OFF
                    )
                    emit_load(eng, b, rb)
                    if i >= 1:
                        emit_store(eng, my_batches[i - 1])
                emit_store(eng, my_batches[-1])
            for b in my_batches:
                s = b % NBUF
                for c in range(C):
                    eng.wait_ge(store_sems[s][c], 16 * (b // NBUF + 1))

        @block.sync
        def _(sync):
            sync.dma_start(
                sb_shifts[0:1, :], shifts.rearrange("b s -> (b s)")[None, :]
            ).then_inc(pre_sem, 16)
            sync.wait_ge(pre_sem, 16)
            emit_half(sync, list(range(0, B // 2)))

        @block.scalar
        def _(scalar):
            scalar.wait_ge(pre_sem, 16)
            emit_half(scalar, list(range(B // 2, B)))

    return nc


_NC_CACHE = None


def _get_nc():
    global _NC_CACHE
    if _NC_CACHE is None:
        _NC_CACHE = build_kernel()
    return _NC_CACHE


def _pad_input(x: np.ndarray) -> np.ndarray:
    """[64, 3, 512, 512] -> [64*3, 296000]: per-channel circular 16-px
    border (544x544) flattened, with 64 tail-pad elems per channel."""
    xp = np.pad(
        x,
        ((0, 0), (0, 0), (MAX_SHIFT, MAX_SHIFT), (MAX_SHIFT, MAX_SHIFT)),
        mode="wrap",
    ).reshape(B_TOTAL * C, CH_ELEMS)
    outp = np.zeros((B_TOTAL * C, CH_STRIDE), dtype=np.float32)
    outp[:, :CH_ELEMS] = xp
    return outp


def kernel(x: np.ndarray, shifts: np.ndarray) -> np.ndarray:
    assert x.shape == (B_TOTAL, C, H, W), x.shape
    assert shifts.shape == (B_TOTAL, 2), shifts.shape
    x = np.ascontiguousarray(x, dtype=np.float32)
    shifts = np.ascontiguousarray(shifts, dtype=np.int32)
    x_pad = _pad_input(x)

    in_maps = [
        {
            "x": x_pad[i * B * C : (i + 1) * B * C],
            "shifts": shifts[i * B : (i + 1) * B],
        }
        for i in range(N_CORES)
    ]
    res = run_bass_kernel_spmd(_get_nc(), in_maps, list(range(N_CORES)))
    return np.concatenate(
        [res.results[i]["out"] for i in range(N_CORES)], axis=0
    ).astype(np.float32)


# revision 19
# speedup vs baseline: 1.1942x; 1.1827x over previous
"""v9: circular-padded input + dynamic-offset full-partition DMAs.

Problem: x [64, 3, 512, 512] f32, shifts [64, 2] int32 in [-16, 16].
out[b, c, h, w] = x[b, c, (h - shifts[b,0]) % 512, (w - shifts[b,1]) % 512]

Pure data parallel over batch (8 per core). Host-side, each channel is
circular-padded by MAX_SHIFT=16 on every border (544x544) and stored as
one flat row of a [24, 296000] tensor (64 tail-pad elems so the widest
dynamic window stays in bounds). BOTH rolls then become a plain window
read at element offset (16-sh)*544 + (16-sw) -- no wraparound pieces,
no If chains. Each (batch, channel) is ONE fixed-shape [128, 4, 512]
load DMA whose DRAM offset is a single register value; the store is a
fully static contiguous DMA.

Why [128, ...] everywhere: the HWDGE fans one DMA's descriptors across
n SDMA engines where n = largest divisor <= 16 of the partition count
(measured: 128 parts -> 16 engines at 341 GB/s; 42 -> 14; 41 (prime) ->
~1 engine at 24 GB/s). The previous design's H-roll pieces had 36-43
partition bodies -- a gcd lottery that serialized ~70% of load bytes on
one engine (761 us total vs ~141 us HBM roofline).

Engine assignment: each dynamic-offset DMA permanently consumes a few
sequencer registers at trace time (offset + bounds-check lowering), and
an engine has only 49 -- 24 dynamic loads on one engine exhausts the
file. So the dynamic loads are split across the two HWDGE sequencers
(sync: batches 0-3, scalar: batches 4-7, own register files) and the
static stores go through gpsimd (SWDGE, measured 377 GB/s on
[128, x, 512] shapes). Offsets are computed with in-place reg ALU ops
on two reused registers (reg_alu with an int immediate also leaks a
const register per call, so 16 and 544 are hoisted into registers).

SBUF layout per slot: [128, 3, 4, 512]: tile[p, c*2048 + j*512 + w]
holds out[b, c, p*4 + j, w].
"""

from contextlib import ExitStack

import numpy as np

import concourse.bass as bass
import concourse.mybir as mybir
from bass_rust import RegisterHandles, make_scalar_value
from concourse.bass_utils import run_bass_kernel_spmd

B_TOTAL, C, H, W = 64, 3, 512, 512
N_CORES = 8
B = B_TOTAL // N_CORES
MAX_SHIFT = 16
PAD = 2 * MAX_SHIFT  # 32
HP, WP = H + PAD, W + PAD  # 544, 544
CH_ELEMS = HP * WP  # 295936
CH_STRIDE = CH_ELEMS + 64  # 296000, tail pad keeps max window in bounds
P = 128
JH = H // P  # 4 rows of a channel per partition
WIN = P * JH * WP  # 278528: window covering 512 padded rows
MAX_OFF = PAD * WP + PAD  # 17440
NBUF = 8  # one slot per batch: no slot reuse, no cross-engine waits

LOADS_PER_BATCH = C
STORES_PER_BATCH = C


def build_kernel():
    nc = bass.Bass()
    x = nc.dram_tensor("x", [B * C, CH_STRIDE], mybir.dt.float32, kind="ExternalInput")
    shifts = nc.dram_tensor("shifts", [B, 2], mybir.dt.int32, kind="ExternalInput")
    out = nc.dram_tensor("out", [B, C, H, W], mybir.dt.float32, kind="ExternalOutput")

    CW = JH * W  # 2048 elems per channel per partition

    with (
        nc.sbuf_tensor([P, NBUF, C * CW], mybir.dt.float32) as tiles,
        nc.sbuf_tensor([1, 2 * B * 2], mybir.dt.int32) as sb_shifts,
        nc.semaphore("pre_sem_sp") as pre_sem_sp,
        nc.semaphore("pre_sem_act") as pre_sem_act,
        ExitStack() as stack,
    ):
        # per (slot, channel) semaphores: stores start as soon as their own
        # 1 MB channel lands (not the whole 3 MB batch), and channel loads
        # only wait for the matching channel's old store -- trims the
        # pipeline ramp and tail by ~2 channels' worth of transfer time.
        load_sems = [
            [
                stack.enter_context(nc.semaphore(f"load_sem{s}_{c}"))
                for c in range(C)
            ]
            for s in range(NBUF)
        ]
        store_sems = [
            [
                stack.enter_context(nc.semaphore(f"store_sem{s}_{c}"))
                for c in range(C)
            ]
            for s in range(NBUF)
        ]
        block = stack.enter_context(nc.Block())

        def emit_load(eng, b, rb):
            s = b % NBUF
            tile_s = tiles[:, s]
            for c in range(C):
                if b >= NBUF:
                    eng.wait_ge(store_sems[s][c], 16 * (b // NBUF))
                win = x[b * C + c, bass.ds(rb, WIN)]
                src = win.rearrange("(p j w) -> p j w", j=JH, w=WP)[:, :, 0:W]
                eng.dma_start(tile_s[:, c * CW : (c + 1) * CW], src).then_inc(
                    load_sems[s][c], 16
                )

        def emit_store(eng, b):
            s = b % NBUF
            tile_s = tiles[:, s]
            for c in range(C):
                eng.wait_ge(load_sems[s][c], 16 * (b // NBUF + 1))
                eng.dma_start(
                    out[b, c].rearrange("(p j) w -> p (j w)", j=JH),
                    tile_s[:, c * CW : (c + 1) * CW],
                ).then_inc(store_sems[s][c], 16)

        def emit_half(eng, my_batches, sh_base, my_pre_target):
            # Interleave loads and stores on ONE engine/ring: SDMA engines
            # round-robin between queues at packet granularity, so the
            # load:store bandwidth split matches the queue mix. Two load
            # queues + one store queue (v10) gave loads 2/3 of bandwidth:
            # all loads finished at t=100us, stores drained alone for 45us.
            # Stores are issued BEFORE the next batch's loads (per channel):
            # with loads-first, the ring still held TWO batches of store
            # descriptors behind the last load -- a ~26us store-only tail.
            with (
                eng.register("r_off") as r_off,
                eng.register("r_sw") as r_sw,
                eng.register("r_cB") as r_cB,
            ):
                # r_off = (16-sh)*544 + (16-sw) = 8720 - (sh*544 + sw)
                eng.reg_mov(r_cB, MAX_SHIFT * WP + MAX_SHIFT)
                for i, b in enumerate(my_batches):
                    eng.reg_load(
                        [r_off, r_sw], sb_shifts[0:1, sh_base + 2 * b : sh_base + 2 * b + 2]
                    )
                    eng.reg_mul(r_off, r_off, WP)
                    eng.reg_add(r_off, r_off, r_sw)
                    eng.reg_sub(r_off, r_cB, r_off)
                    rb = make_scalar_value(
                        RegisterHandles([r_off]), min_val=0, max_val=MAX_OFF
                    )
                    s = b % NBUF
                    tile_s = tiles[:, s]
                    bp = my_batches[i - 1] if i >= 1 else None
                    for c in range(C):
                        if bp is not None:
                            sp = bp % NBUF
                            eng.wait_ge(load_sems[sp][c], 16 * (bp // NBUF + 1))
                            eng.dma_start(
                                out[bp, c].rearrange("(p j) w -> p (j w)", j=JH),
                                tiles[:, sp][:, c * CW : (c + 1) * CW],
                            ).then_inc(store_sems[sp][c], 16)
                        if b >= NBUF:
                            eng.wait_ge(store_sems[s][c], 16 * (b // NBUF))
                        win = x[b * C + c, bass.ds(rb, WIN)]
                        src = win.rearrange("(p j w) -> p j w", j=JH, w=WP)[
                            :, :, 0:W
                        ]
                        eng.dma_start(
                            tile_s[:, c * CW : (c + 1) * CW], src
                        ).then_inc(load_sems[s][c], 16)
                emit_store(eng, my_batches[-1])
            for b in my_batches:
                s = b % NBUF
                for c in range(C):
                    eng.wait_ge(store_sems[s][c], 16 * (b // NBUF + 1))

        def emit_preload(eng, sem, sh_base):
            # each engine preloads its own copy of the shifts with its own
            # semaphore: no cross-engine dependency on the ramp
            eng.dma_start(
                sb_shifts[0:1, sh_base : sh_base + 2 * B],
                shifts.rearrange("b s -> (b s)")[None, :],
            ).then_inc(sem, 16)
            eng.wait_ge(sem, 16)

        @block.sync
        def _(sync):
            emit_preload(sync, pre_sem_sp, 0)
            emit_half(sync, list(range(0, B // 2)), 0, 16)

        @block.scalar
        def _(scalar):
            emit_preload(scalar, pre_sem_act, 2 * B)
            emit_half(scalar, list(range(B // 2, B)), 2 * B, 16)

    return nc


_NC_CACHE = None


def _get_nc():
    global _NC_CACHE
    if _NC_CACHE is None:
        _NC_CACHE = build_kernel()
    return _NC_CACHE


def _pad_input(x: np.ndarray) -> np.ndarray:
    """[64, 3, 512, 512] -> [64*3, 296000]: per-channel circular 16-px
    border (544x544) flattened, with 64 tail-pad elems per channel."""
    xp = np.pad(
        x,
        ((0, 0), (0, 0), (MAX_SHIFT, MAX_SHIFT), (MAX_SHIFT, MAX_SHIFT)),
        mode="wrap",
    ).reshape(B_TOTAL * C, CH_ELEMS)
    outp = np.zeros((B_TOTAL * C, CH_STRIDE), dtype=np.float32)
    outp[:, :CH_ELEMS] = xp
    return outp


def kernel(x: np.ndarray, shifts: np.ndarray) -> np.ndarray:
    assert x.shape == (B_TOTAL, C, H, W), x.shape
    assert shifts.shape == (B_TOTAL, 2), shifts.shape
    x = np.ascontiguousarray(x, dtype=np.float32)
    shifts = np.ascontiguousarray(shifts, dtype=np.int32)
    x_pad = _pad_input(x)

    in_maps = [
        {
            "x": x_pad[i * B * C : (i + 1) * B * C],
            "shifts": shifts[i * B : (i + 1) * B],
        }
        for i in range(N_CORES)
    ]
    res = run_bass_kernel_spmd(_get_nc(), in_maps, list(range(N_CORES)))
    return np.concatenate(
        [res.results[i]["out"] for i in range(N_CORES)], axis=0
    ).astype(np.float32)


# revision 20
# speedup vs baseline: 1.7076x; 1.4299x over previous
"""v15: direct HBM->HBM rolled-window copies (no SBUF staging).

Problem: x [64, 3, 512, 512] f32, shifts [64, 2] int32 in [-16, 16].
out[b, c, h, w] = x[b, c, (h - shifts[b,0]) % 512, (w - shifts[b,1]) % 512]

Host-side, each channel is circular-padded by 16 px (544x544, flat rows
of a [24, 296000] tensor). Both rolls collapse into a window read at
element offset (16-sh)*544 + (16-sw). v14 staged windows through SBUF
(load + store), pushing 2x the bytes through the SDMA engines and
saturating the 436 GB/s SBUF-fabric ceiling at 96%. Here each (batch,
channel) is ONE DRAM->DRAM DMA: out[b,c] (contiguous [512,512]) <-
padded window ([512 rows @ 544 stride, 512 cols], dynamic offset).
Equal dim-0 counts (512 rows both sides) hit the DGE's HbmToHbm
reshape: descriptors fan across all 16 SDMA engines, and each byte
crosses an engine once instead of twice.

No SBUF tiles, no slot semaphores -- batches are fully independent;
each engine just drains its 12 copy DMAs and waits for its completion
count. Dynamic-offset DMAs leak sequencer registers at trace time
(49/engine budget), so the 24 copies split across the two HWDGE
sequencers, offsets computed with in-place reg ALU on reused registers
(constants hoisted; see memory notes).
"""

from contextlib import ExitStack

import numpy as np

import concourse.bass as bass
import concourse.mybir as mybir
from bass_rust import RegisterHandles, make_scalar_value
from concourse.bass_utils import run_bass_kernel_spmd

B_TOTAL, C, H, W = 64, 3, 512, 512
N_CORES = 8
B = B_TOTAL // N_CORES
MAX_SHIFT = 16
PAD = 2 * MAX_SHIFT  # 32
HP, WP = H + PAD, W + PAD  # 544, 544
CH_ELEMS = HP * WP  # 295936
CH_STRIDE = CH_ELEMS + 64  # 296000, tail pad keeps max window in bounds
WIN = H * WP  # 278528: window covering 512 padded rows
MAX_OFF = PAD * WP + PAD  # 17440


def build_kernel():
    nc = bass.Bass()
    x = nc.dram_tensor("x", [B * C, CH_STRIDE], mybir.dt.float32, kind="ExternalInput")
    shifts = nc.dram_tensor("shifts", [B, 2], mybir.dt.int32, kind="ExternalInput")
    out = nc.dram_tensor("out", [B, C, H, W], mybir.dt.float32, kind="ExternalOutput")

    with (
        nc.sbuf_tensor([1, 2 * B * 2], mybir.dt.int32) as sb_shifts,
        nc.semaphore("pre_sem_sp") as pre_sem_sp,
        nc.semaphore("pre_sem_act") as pre_sem_act,
        nc.semaphore("done_sp") as done_sp,
        nc.semaphore("done_act") as done_act,
        ExitStack() as stack,
    ):
        block = stack.enter_context(nc.Block())

        def emit_half(eng, my_batches, sh_base, pre_sem, done_sem):
            eng.dma_start(
                sb_shifts[0:1, sh_base : sh_base + 2 * B],
                shifts.rearrange("b s -> (b s)")[None, :],
            ).then_inc(pre_sem, 16)
            eng.wait_ge(pre_sem, 16)
            n = 0
            with (
                eng.register("r_off") as r_off,
                eng.register("r_sw") as r_sw,
                eng.register("r_cB") as r_cB,
            ):
                # r_off = (16-sh)*544 + (16-sw) = 8720 - (sh*544 + sw)
                eng.reg_mov(r_cB, MAX_SHIFT * WP + MAX_SHIFT)
                for b in my_batches:
                    eng.reg_load(
                        [r_off, r_sw],
                        sb_shifts[0:1, sh_base + 2 * b : sh_base + 2 * b + 2],
                    )
                    eng.reg_mul(r_off, r_off, WP)
                    eng.reg_add(r_off, r_off, r_sw)
                    eng.reg_sub(r_off, r_cB, r_off)
                    rb = make_scalar_value(
                        RegisterHandles([r_off]), min_val=0, max_val=MAX_OFF
                    )
                    for c in range(C):
                        win = x[b * C + c, bass.ds(rb, WIN)]
                        src = win.rearrange("(r w) -> r w", w=WP)[:, 0:W]
                        eng.dma_start(out[b, c], src).then_inc(done_sem, 16)
                        n += 1
            eng.wait_ge(done_sem, 16 * n)

        @block.sync
        def _(sync):
            emit_half(sync, list(range(0, B // 2)), 0, pre_sem_sp, done_sp)

        @block.scalar
        def _(scalar):
            emit_half(scalar, list(range(B // 2, B)), 2 * B, pre_sem_act, done_act)

    return nc


_NC_CACHE = None


def _get_nc():
    global _NC_CACHE
    if _NC_CACHE is None:
        _NC_CACHE = build_kernel()
    return _NC_CACHE


def _pad_input(x: np.ndarray) -> np.ndarray:
    """[64, 3, 512, 512] -> [64*3, 296000]: per-channel circular 16-px
    border (544x544) flattened, with 64 tail-pad elems per channel."""
    xp = np.pad(
        x,
        ((0, 0), (0, 0), (MAX_SHIFT, MAX_SHIFT), (MAX_SHIFT, MAX_SHIFT)),
        mode="wrap",
    ).reshape(B_TOTAL * C, CH_ELEMS)
    outp = np.zeros((B_TOTAL * C, CH_STRIDE), dtype=np.float32)
    outp[:, :CH_ELEMS] = xp
    return outp


def kernel(x: np.ndarray, shifts: np.ndarray) -> np.ndarray:
    assert x.shape == (B_TOTAL, C, H, W), x.shape
    assert shifts.shape == (B_TOTAL, 2), shifts.shape
    x = np.ascontiguousarray(x, dtype=np.float32)
    shifts = np.ascontiguousarray(shifts, dtype=np.int32)
    x_pad = _pad_input(x)

    in_maps = [
        {
            "x": x_pad[i * B * C : (i + 1) * B * C],
            "shifts": shifts[i * B : (i + 1) * B],
        }
        for i in range(N_CORES)
    ]
    res = run_bass_kernel_spmd(_get_nc(), in_maps, list(range(N_CORES)))
    return np.concatenate(
        [res.results[i]["out"] for i in range(N_CORES)], axis=0
    ).astype(np.float32)


# revision 24
# speedup vs baseline: 1.7555x; 1.0280x over previous
"""v15: direct HBM->HBM rolled-window copies (no SBUF staging).

Problem: x [64, 3, 512, 512] f32, shifts [64, 2] int32 in [-16, 16].
out[b, c, h, w] = x[b, c, (h - shifts[b,0]) % 512, (w - shifts[b,1]) % 512]

Host-side, each channel is circular-padded by 16 px (544x544, flat rows
of a [24, 296000] tensor). Both rolls collapse into a window read at
element offset (16-sh)*544 + (16-sw). v14 staged windows through SBUF
(load + store), pushing 2x the bytes through the SDMA engines and
saturating the 436 GB/s SBUF-fabric ceiling at 96%. Here each (batch,
channel) is ONE DRAM->DRAM DMA: out[b,c] (contiguous [512,512]) <-
padded window ([512 rows @ 544 stride, 512 cols], dynamic offset).
Equal dim-0 counts (512 rows both sides) hit the DGE's HbmToHbm
reshape: descriptors fan across all 16 SDMA engines, and each byte
crosses an engine once instead of twice.

No SBUF tiles, no slot semaphores -- batches are fully independent;
each engine just drains its 12 copy DMAs and waits for its completion
count. Dynamic-offset DMAs leak sequencer registers at trace time
(49/engine budget), so the 24 copies split across the two HWDGE
sequencers, offsets computed with in-place reg ALU on reused registers
(constants hoisted; see memory notes).
"""

from contextlib import ExitStack

import numpy as np

import concourse.bass as bass
import concourse.mybir as mybir
from bass_rust import RegisterHandles, make_scalar_value
from concourse.bass_utils import run_bass_kernel_spmd

B_TOTAL, C, H, W = 64, 3, 512, 512
N_CORES = 8
B = B_TOTAL // N_CORES
MAX_SHIFT = 16
PAD = 2 * MAX_SHIFT  # 32
HP, WP = H + PAD, W + PAD  # 544, 544
CH_ELEMS = HP * WP  # 295936
CH_STRIDE = CH_ELEMS + 64  # 296000, tail pad keeps max window in bounds
WIN = H * WP  # 278528: window covering 512 padded rows
MAX_OFF = PAD * WP + PAD  # 17440


def build_kernel():
    nc = bass.Bass()
    x = nc.dram_tensor("x", [B * C, CH_STRIDE], mybir.dt.float32, kind="ExternalInput")
    shifts = nc.dram_tensor("shifts", [B, 2], mybir.dt.int32, kind="ExternalInput")
    out = nc.dram_tensor("out", [B, C, H, W], mybir.dt.float32, kind="ExternalOutput")

    with (
        nc.sbuf_tensor([1, 3 * B * 2], mybir.dt.int32) as sb_shifts,
        nc.semaphore("pre_sem_sp") as pre_sem_sp,
        nc.semaphore("pre_sem_act") as pre_sem_act,
        nc.semaphore("pre_sem_gp") as pre_sem_gp,
        nc.semaphore("done_sp") as done_sp,
        nc.semaphore("done_act") as done_act,
        nc.semaphore("done_gp") as done_gp,
        ExitStack() as stack,
    ):
        block = stack.enter_context(nc.Block())

        def emit_half(eng, my_batches, sh_base, pre_sem, done_sem):
            eng.dma_start(
                sb_shifts[0:1, sh_base : sh_base + 2 * B],
                shifts.rearrange("b s -> (b s)")[None, :],
            ).then_inc(pre_sem, 16)
            eng.wait_ge(pre_sem, 16)
            n = 0
            with (
                eng.register("r_off") as r_off,
                eng.register("r_sw") as r_sw,
                eng.register("r_cB") as r_cB,
            ):
                # r_off = (16-sh)*544 + (16-sw) = 8720 - (sh*544 + sw)
                eng.reg_mov(r_cB, MAX_SHIFT * WP + MAX_SHIFT)
                for b in my_batches:
                    eng.reg_load(
                        [r_off, r_sw],
                        sb_shifts[0:1, sh_base + 2 * b : sh_base + 2 * b + 2],
                    )
                    eng.reg_mul(r_off, r_off, WP)
                    eng.reg_add(r_off, r_off, r_sw)
                    eng.reg_sub(r_off, r_cB, r_off)
                    rb = make_scalar_value(
                        RegisterHandles([r_off]), min_val=0, max_val=MAX_OFF
                    )
                    for c in range(C):
                        win = x[b * C + c, bass.ds(rb, WIN)]
                        src = win.rearrange("(r w) -> r w", w=WP)[:, 0:W]
                        eng.dma_start(out[b, c], src).then_inc(done_sem, 16)
                        n += 1
            eng.wait_ge(done_sem, 16 * n)

        # three issue queues (qSync, qScalar, qGpSimd): SDMA engines switch
        # queue contexts at packet boundaries, so packets from a third queue
        # can overlap another queue's per-packet gap
        @block.sync
        def _(sync):
            emit_half(sync, [0, 1, 2], 0, pre_sem_sp, done_sp)

        @block.scalar
        def _(scalar):
            emit_half(scalar, [3, 4, 5], 2 * B, pre_sem_act, done_act)

        @block.gpsimd
        def _(gp):
            emit_half(gp, [6, 7], 4 * B, pre_sem_gp, done_gp)

    return nc


_NC_CACHE = None


def _get_nc():
    global _NC_CACHE
    if _NC_CACHE is None:
        _NC_CACHE = build_kernel()
    return _NC_CACHE


def _pad_input(x: np.ndarray) -> np.ndarray:
    """[64, 3, 512, 512] -> [64*3, 296000]: per-channel circular 16-px
    border (544x544) flattened, with 64 tail-pad elems per channel."""
    xp = np.pad(
        x,
        ((0, 0), (0, 0), (MAX_SHIFT, MAX_SHIFT), (MAX_SHIFT, MAX_SHIFT)),
        mode="wrap",
    ).reshape(B_TOTAL * C, CH_ELEMS)
    outp = np.zeros((B_TOTAL * C, CH_STRIDE), dtype=np.float32)
    outp[:, :CH_ELEMS] = xp
    return outp


def kernel(x: np.ndarray, shifts: np.ndarray) -> np.ndarray:
    assert x.shape == (B_TOTAL, C, H, W), x.shape
    assert shifts.shape == (B_TOTAL, 2), shifts.shape
    x = np.ascontiguousarray(x, dtype=np.float32)
    shifts = np.ascontiguousarray(shifts, dtype=np.int32)
    x_pad = _pad_input(x)

    in_maps = [
        {
            "x": x_pad[i * B * C : (i + 1) * B * C],
            "shifts": shifts[i * B : (i + 1) * B],
        }
        for i in range(N_CORES)
    ]
    res = run_bass_kernel_spmd(_get_nc(), in_maps, list(range(N_CORES)))
    return np.concatenate(
        [res.results[i]["out"] for i in range(N_CORES)], axis=0
    ).astype(np.float32)
